# revision 3
# baseline (speedup 1.0000x reference)
"""CoAttention Trainium2 kernel, v2: fp8 DoubleRow attention.

Problem: B=16, PLEN=1024, QLEN=256, D=256 fp32, data-parallel over batch
on 8 cores (2 batches/core). Per batch:

  score[p,q] = s0[p,q] + sp[p] + sq[q] + b,  s0 = (P*w_pq) @ Q^T
  p2q = rowsoftmax(score)*keep; q2p = colsoftmax(score)*keep^T
  outputs: p2q@Q and p2q@(q2p@P)   (q2p@P itself is not an output)

Factorization (all row/col-constant factors cancel inside softmax):
  E = exp(s0)            [p,q]  fp8      (range checked: |s0| < 4.5)
  h = exp(sp - 1e7*pm)   [p]             (p-mask as exact exp zero)
  g = exp(sq + b - 1e7*qm) [q]
  q2p' = diag(g/(E^T.h)) E^T diag(h) P      -> rhs for coatt
  p2q  = diag(kp/(E.g))  E  diag(g) Q
  co   = diag(kp/(E.g))  E @ q2p'

PE work: scores in fp32r (PT via PE transpose, w_pq folded into the
PT psum->sbuf drain); E transposed on PE in fp8 (step-2 output layout);
the three attention matmuls run as fp8e4 DoubleRow (two K-tiles per
pass, 0.5 cyc/row): rhs operands P*h, Q*g, q2p' are written as fp8
by the Pool engine (h/g scaling rides the conversion op).

Engine split: PE matmuls; ACT exps + PT drains; DVE qt/ET/q2p drains;
Pool fp8 conversions (SBUF only - Pool cannot access PSUM); output
scale-drains split DVE/ACT; DMA spread across SP/Pool/ACT/DVE queues.
"""

import numpy as np
import orjson

import concourse.bass as bass
import concourse.mybir as mybir
import concourse.tile as tile
from concourse.bass_utils import run_bass_kernel_spmd
from concourse.masks import make_identity

F32 = mybir.dt.float32
F32R = mybir.dt.float32r
BF16 = mybir.dt.bfloat16
FP8 = mybir.dt.float8e4
I32 = mybir.dt.int32
AF = mybir.ActivationFunctionType
DR = mybir.MatmulPerfMode.DoubleRow
MUL = mybir.AluOpType.mult
ADD = mybir.AluOpType.add

N_CORES = 8
B, PLEN, QLEN, D = 16, 1024, 256, 256
NB = B // N_CORES
PT_T = PLEN // 128  # 8
QT_T = QLEN // 128  # 2
DT_T = D // 128  # 2
MASK = -10000000.0
EPS = 1e-30

# ---------------------------------------------------------------------------
# walrus single-wait workaround (split multi-waits into EventSemaphores)


def _split_waits_in_bir(bir: dict) -> None:
    for f in bir.get("functions", []):
        for blk in f.get("blocks", []):
            out = []
            for i in blk.get("instructions", []):
                si = i.get("sync_info")
                ow = (si or {}).get("on_wait") or []
                limit = 0 if i.get("opcode") == "Matmult" else 1
                if len(ow) > limit:
                    for k, w in enumerate(ow[limit:]):
                        out.append(
                            {
                                "debug": i.get("debug"),
                                "engine": i["engine"],
                                "ins": [],
                                "outs": [],
                                "name": f"{i['name']}__w{k}",
                                "opcode": "EventSemaphore",
                                "sync_info": {"on_update": [], "on_wait": [w]},
                            }
                        )
                    si["on_wait"] = ow[:limit]
                out.append(i)
            blk["instructions"] = out


_patched = False


def _install_bir_wait_split():
    global _patched
    if _patched:
        return
    _patched = True
    import concourse.bass2jax as b2j
    import concourse.bass_utils as bu

    orig = bu.compile_bir_kernel

    def patched(bir_json, tmpdir, neff_name="file.neff"):
        bir = orjson.loads(bir_json)
        _split_waits_in_bir(bir)
        return orig(orjson.dumps(bir), tmpdir, neff_name)

    bu.compile_bir_kernel = patched
    b2j.compile_bir_kernel = patched


# ---------------------------------------------------------------------------


def build_nc() -> bass.Bass:
    nc = bass.Bass()
    passage = nc.declare_dram_parameter("passage", [NB, PLEN, D], F32, isOutput=False)
    question = nc.declare_dram_parameter("question", [NB, QLEN, D], F32, isOutput=False)
    pmask = nc.declare_dram_parameter("passage_mask", [NB, PLEN], I32, isOutput=False)
    qmask = nc.declare_dram_parameter("question_mask", [NB, QLEN], I32, isOutput=False)
    w_all = nc.declare_dram_parameter("W", [3 * D], F32, isOutput=False)
    b_in = nc.declare_dram_parameter("b", [1], F32, isOutput=False)
    out_p2q = nc.declare_dram_parameter("p2q", [NB, PLEN, D], F32, isOutput=True)
    out_co = nc.declare_dram_parameter("coatt", [NB, PLEN, D], F32, isOutput=True)

    with tile.TileContext(nc) as tc:
        with (
            tc.tile_pool(name="const", bufs=1) as cp,
            tc.tile_pool(name="sb", bufs=2) as sb,
            tc.tile_pool(name="tp_ps", bufs=2, space="PSUM") as tp_ps,
            tc.tile_pool(name="s0_ps", bufs=2, space="PSUM") as s0_ps,
            tc.tile_pool(name="et_ps", bufs=2, space="PSUM") as et_ps,
            tc.tile_pool(name="at_ps", bufs=2, space="PSUM") as at_ps,
        ):
            # ---- constants -------------------------------------------------
            ident = cp.tile([128, 128], F32, name="ident")
            make_identity(nc, ident[:])
            ident_r = cp.tile([128, 128], F32R, name="ident_r")
            nc.vector.tensor_copy(ident_r[:], ident[:])
            identb = cp.tile([128, 128], BF16, name="identb")
            nc.vector.tensor_copy(identb[:], ident[:])

            # preload the exp act table off the critical path
            dummy = cp.tile([128, 1], F32, name="dummy")
            nc.scalar.activation(dummy[:], ident[:, 0:1], AF.Exp)

            # tile declarations (DMAs and prep ops are emitted below, in
            # program order after the loads that fill them)
            w6 = cp.tile([128, 6], F32, name="w6")
            w_pq = w6[:, 2 * DT_T : 3 * DT_T]
            w_q_r = cp.tile([128, DT_T, 2], F32R, name="w_q_r")
            wr_p = cp.tile([128, DT_T, 2], F32R, name="wr_p")
            winv = cp.tile([128, DT_T], F32, name="winv")
            b_sb = cp.tile([128, 1], F32, name="b_sb")
            pm_all = cp.tile([128, NB, PT_T], I32, name="pm_all")
            qm_all = cp.tile([128, NB, QT_T], I32, name="qm_all")
            pm_f = cp.tile([128, NB, PT_T], F32, name="pm_f")
            pmb7 = cp.tile([128, NB, PT_T], F32, name="pmb7")
            kp_all = cp.tile([128, NB, PT_T], F32, name="kp_all")
            qm_f = cp.tile([128, NB, QT_T], F32, name="qm_f")
            qmb = cp.tile([128, NB, QT_T], F32, name="qmb")

            # ---- input loads (both batches up front) ----------------------
            q_sbs, p_sbs = [], []
            for bi in range(NB):
                q_sb = cp.tile([128, QT_T, D], F32R, name=f"q_sb{bi}")
                q_src = question[bi].rearrange("(t q) d -> q t d", q=128).bitcast(F32R)
                for t4 in range(QT_T):
                    nc.sync.dma_start(q_sb[:, t4 : t4 + 1, :], q_src[:, t4 : t4 + 1, :])
                q_sbs.append(q_sb)
                if bi == 0:
                    nc.sync.dma_start(
                        pm_all[:],
                        pmask[:].rearrange("n (t p) -> p n t", p=128),
                    )
                    nc.scalar.dma_start(
                        w6[:], w_all[:].rearrange("(k d) -> d k", d=128)
                    )
            nc.scalar.dma_start(b_sb[:], b_in[0:1].partition_broadcast(128))
            nc.scalar.dma_start(
                qm_all[:], qmask[:].rearrange("n (t q) -> q n t", q=128)
            )
            for bi in range(NB):
                p_sb = cp.tile([128, PT_T, D], F32R, name=f"p_sb{bi}")
                p_src = passage[bi].rearrange("(t p) d -> p t d", p=128).bitcast(F32R)
                for c4 in range(4):
                    nc.gpsimd.dma_start(
                        p_sb[:, c4 * 2 : (c4 + 1) * 2, :],
                        p_src[:, c4 * 2 : (c4 + 1) * 2, :],
                    )
                p_sbs.append(p_sb)

            # ---- weight / mask prep (after their DMAs in program order) ---
            for j in range(DT_T):
                for c in range(2):
                    nc.vector.tensor_copy(
                        w_q_r[:, j, c : c + 1], w6[:, DT_T + j : DT_T + j + 1]
                    )
            nc.vector.reciprocal(winv[:], w_pq[:])
            for j in range(DT_T):
                t0 = cp.tile([128, 1], F32, name=f"wrp{j}")
                nc.vector.tensor_mul(t0[:], w6[:, j : j + 1], winv[:, j : j + 1])
                for c in range(2):
                    nc.vector.tensor_copy(wr_p[:, j, c : c + 1], t0[:])
            nc.vector.tensor_copy(pm_f[:], pm_all[:])
            nc.vector.tensor_scalar_mul(pmb7[:], pm_f[:], MASK)
            nc.vector.tensor_scalar(kp_all[:], pm_f[:], -1.0, 1.0, MUL, ADD)
            nc.vector.tensor_copy(qm_f[:], qm_all[:])
            nc.vector.tensor_scalar(
                qmb[:], qm_f[:], MASK, b_sb[:, 0:1], MUL, ADD
            )

            def emit_batch(bi):
                q_sb, p_sb = q_sbs[bi], p_sbs[bi]
                p2q_dst = out_p2q[bi].rearrange("(t p) d -> p t d", p=128)
                co_dst = out_co[bi].rearrange("(t p) d -> p t d", p=128)

                qt = sb.tile([128, DT_T, QLEN], F32R, name="qt", tag="qt")
                pt_r = sb.tile([128, DT_T, PLEN], F32R, name="pt_r", tag="pt_r")
                e8 = sb.tile([128, PT_T, QLEN], BF16, name="e8", tag="e8")
                et8 = sb.tile([128, QT_T, PLEN], BF16, name="et8", tag="et8")
                p8 = sb.tile([128, PT_T, D + 2], BF16, name="p8", tag="p8")
                qgg8 = sb.tile([128, QT_T, D + 2], BF16, name="qgg8", tag="qgg8")
                q2p8 = sb.tile([128, QT_T, D], BF16, name="q2p8", tag="q2p8")
                g_t = sb.tile([128, QT_T], F32, name="g_t", tag="g_t")
                h_t = sb.tile([128, PT_T], F32, name="h_t", tag="h_t")
                sp2 = sb.tile([128, PT_T], F32, name="sp2", tag="sp2")
                s_vec = sb.tile([128, QT_T], F32, name="s_vec", tag="s_vec")
                rp = sb.tile([128, PT_T], F32, name="rp", tag="rp")
                p2q_sb = sb.tile([128, PT_T, D], F32, name="p2q_sb", tag="p2q_sb")
                co_sb = sb.tile([128, PT_T, D], F32, name="co_sb", tag="co_sb")

                # ---- Q transposes + qt drains (plain; w_pq rides PT side) --
                for j in range(DT_T):
                    tqp = tp_ps.tile([128, 512], F32R, name="tqp", tag="tp")
                    for t4 in range(QT_T):
                        nc.tensor.transpose(
                            tqp[:, t4 * 128 : (t4 + 1) * 128],
                            q_sb[:, t4, j * 128 : (j + 1) * 128],
                            ident_r[:],
                        )
                    nc.vector.tensor_copy(qt[:, j, :], tqp[:, 0:QLEN])

                # sq -> g (exp bias folds -1e7*qm + b)
                sq_ps = at_ps.tile([128, QT_T, 2], F32, name="sq_ps", tag="at")
                for tq in range(QT_T):
                    for j in range(DT_T):
                        nc.tensor.matmul(
                            sq_ps[:, tq, :],
                            qt[:, j, tq * 128 : (tq + 1) * 128],
                            w_q_r[:, j, :],
                            start=(j == 0),
                            stop=(j == DT_T - 1),
                        )
                for tq in range(QT_T):
                    nc.scalar.activation(
                        g_t[:, tq : tq + 1],
                        sq_ps[:, tq, 0:1],
                        AF.Exp,
                        bias=qmb[:, bi, tq : tq + 1],
                    )
                # qgg8 = [Q*g | g | g] in fp8 (Pool)
                for tq in range(QT_T):
                    nc.gpsimd.tensor_scalar_mul(
                        qgg8[:, tq, 0:D], q_sb[:, tq, :].bitcast(F32), g_t[:, tq : tq + 1]
                    )
                nc.gpsimd.tensor_copy(qgg8[:, :, D : D + 1], g_t[:].unsqueeze(-1))
                nc.gpsimd.tensor_copy(qgg8[:, :, D + 1 : D + 2], g_t[:].unsqueeze(-1))
                yield  # head done

                def emit_half(grp):
                    t_lo = grp * 4
                    # PT transposes; drain folds *w_pq (per-partition scale)
                    for j in range(DT_T):
                        tpp = tp_ps.tile([128, 512], F32R, name="tpp", tag="tp")
                        for t4 in range(4):
                            t = t_lo + t4
                            nc.tensor.transpose(
                                tpp[:, t4 * 128 : (t4 + 1) * 128],
                                p_sb[:, t, j * 128 : (j + 1) * 128],
                                ident_r[:],
                            )
                        if j == 0:
                            nc.scalar.activation(
                                pt_r[:, j, grp * 512 : (grp + 1) * 512],
                                tpp[:],
                                AF.Copy,
                                scale=w_pq[:, j : j + 1],
                            )
                        else:
                            nc.vector.tensor_scalar_mul(
                                pt_r[:, j, grp * 512 : (grp + 1) * 512],
                                tpp[:],
                                w_pq[:, j : j + 1],
                            )
                    # sp (off scaled PT, via w_p/w_pq cols) -> h
                    sp_ps = at_ps.tile([128, PT_T, 2], F32, name="sp_ps", tag="at")
                    for t4 in range(4):
                        t = t_lo + t4
                        for j in range(DT_T):
                            nc.tensor.matmul(
                                sp_ps[:, t, :],
                                pt_r[:, j, t * 128 : (t + 1) * 128],
                                wr_p[:, j, :],
                                start=(j == 0),
                                stop=(j == DT_T - 1),
                            )
                    nc.vector.tensor_add(
                        sp2[:, t_lo : t_lo + 4],
                        sp_ps[:, t_lo : t_lo + 4, 0],
                        pmb7[:, bi, t_lo : t_lo + 4],
                    )
                    nc.scalar.activation(
                        h_t[:, t_lo : t_lo + 4], sp2[:, t_lo : t_lo + 4], AF.Exp
                    )
                    # p8 = [P*h | h | h] fp8 (Pool, SBUF only)
                    for t4 in range(4):
                        t = t_lo + t4
                        nc.gpsimd.tensor_scalar_mul(
                            p8[:, t, 0:D], p_sb[:, t, :].bitcast(F32), h_t[:, t : t + 1]
                        )
                    nc.gpsimd.tensor_copy(
                        p8[:, t_lo : t_lo + 4, D : D + 1],
                        h_t[:, t_lo : t_lo + 4].unsqueeze(-1),
                    )
                    nc.gpsimd.tensor_copy(
                        p8[:, t_lo : t_lo + 4, D + 1 : D + 2],
                        h_t[:, t_lo : t_lo + 4].unsqueeze(-1),
                    )
                    # S0 pairs + exp -> e8
                    for i in range(2):
                        t0 = t_lo + i * 2
                        s0p = s0_ps.tile([128, 2, 256], F32, name="s0p", tag="s0")
                        for t4 in range(2):
                            t = t0 + t4
                            for j in range(DT_T):
                                nc.tensor.matmul(
                                    s0p[:, t4, :],
                                    pt_r[:, j, t * 128 : (t + 1) * 128],
                                    qt[:, j, :],
                                    start=(j == 0),
                                    stop=(j == DT_T - 1),
                                )
                        nc.scalar.activation(
                            e8[:, t0 : t0 + 2, :], s0p[:], AF.Exp
                        )
                    # E transposes (bf16) for this half's p cols
                    for tq in range(QT_T):
                        etp = et_ps.tile([128, 512], BF16, name="etp", tag="et")
                        for t4 in range(4):
                            nc.tensor.transpose(
                                etp[:, t4 * 128 : (t4 + 1) * 128],
                                e8[:, t_lo + t4, tq * 128 : (tq + 1) * 128],
                                identb[:],
                            )
                        # packed bf16 psum -> sbuf copy (DVE 2x mode)
                        nc.vector.tensor_copy(
                            et8[:, tq, grp * 512 : (grp + 1) * 512],
                            etp[:],
                        )

                def emit_p2q_pair(tp, act=False):
                    t0 = 2 * tp
                    app = at_ps.tile([128, 2, D], F32, name="app", tag="at")
                    den = s0_ps.tile([128, 2, 2], F32, name="den", tag="s0")
                    for t4 in range(2):
                        t = t0 + t4
                        for tq in range(QT_T):
                            nc.tensor.matmul(
                                app[:, t4, :],
                                et8[:, tq, t * 128 : (t + 1) * 128],
                                qgg8[:, tq, 0:D],
                                start=(tq == 0), stop=(tq == QT_T - 1),
                            )
                        for tq in range(QT_T):
                            nc.tensor.matmul(
                                den[:, t4, :],
                                et8[:, tq, t * 128 : (t + 1) * 128],
                                qgg8[:, tq, D : D + 2],
                                start=(tq == 0), stop=(tq == QT_T - 1),
                            )
                    v2 = sb.tile([128, 2], F32, name="v2", tag="v2")
                    nc.vector.reciprocal(v2[:], den[:, :, 0])
                    nc.vector.tensor_mul(
                        rp[:, t0 : t0 + 2], v2[:], kp_all[:, bi, t0 : t0 + 2]
                    )
                    if act:
                        for t4 in range(2):
                            nc.scalar.activation(
                                p2q_sb[:, t0 + t4, :], app[:, t4, :], AF.Copy,
                                scale=rp[:, t0 + t4 : t0 + t4 + 1],
                            )
                    else:
                        nc.vector.tensor_mul(
                            p2q_sb[:, t0 : t0 + 2, :],
                            app[:],
                            rp[:, t0 : t0 + 2].unsqueeze(-1).broadcast_to([128, 2, D]),
                        )

                def emit_co_pair(tp, act=False, split=False, et=False):
                    t0 = 2 * tp
                    cop = (
                        et_ps.tile([128, 2, D], F32, name="cop", tag="et")
                        if et
                        else s0_ps.tile([128, 2, D], F32, name="cop", tag="s0")
                    )
                    for t4 in range(2):
                        t = t0 + t4
                        for tq in range(QT_T):
                            nc.tensor.matmul(
                                cop[:, t4, :],
                                et8[:, tq, t * 128 : (t + 1) * 128],
                                q2p8[:, tq, :],
                                start=(tq == 0), stop=(tq == QT_T - 1),
                            )
                    if split:
                        nc.vector.tensor_scalar_mul(
                            co_sb[:, t0, :], cop[:, 0, :], rp[:, t0 : t0 + 1]
                        )
                        nc.scalar.activation(
                            co_sb[:, t0 + 1, :], cop[:, 1, :], AF.Copy,
                            scale=rp[:, t0 + 1 : t0 + 2],
                        )
                    elif act:
                        for t4 in range(2):
                            nc.scalar.activation(
                                co_sb[:, t0 + t4, :], cop[:, t4, :], AF.Copy,
                                scale=rp[:, t0 + t4 : t0 + t4 + 1],
                            )
                    else:
                        nc.vector.tensor_mul(
                            co_sb[:, t0 : t0 + 2, :],
                            cop[:],
                            rp[:, t0 : t0 + 2].unsqueeze(-1).broadcast_to([128, 2, D]),
                        )

                emit_half(0)
                # p2q for half-0 tiles runs while half-1 scores cook
                for tp in range(2):
                    emit_p2q_pair(tp)
                    nc.sync.dma_start(
                        p2q_dst[:, 2 * tp : 2 * tp + 2, :],
                        p2q_sb[:, 2 * tp : 2 * tp + 2, :],
                    )
                yield  # half 0 + early p2q done
                emit_half(1)
                yield  # half 1 done
                # aq (needs all of e8/p8) -> q2p8
                for tq in range(QT_T):
                    aqp = at_ps.tile([128, D + 2], F32, name="aqp", tag="at")
                    for t in range(PT_T):
                        nc.tensor.matmul(
                            aqp[:],
                            e8[:, t, tq * 128 : (tq + 1) * 128],
                            p8[:, t, :],
                            start=(t == 0),
                            stop=(t == PT_T - 1),
                        )
                    u2 = sb.tile([128, 1], F32, name="u2", tag="u2")
                    nc.vector.reciprocal(u2[:], aqp[:, D : D + 1])
                    nc.vector.tensor_mul(
                        s_vec[:, tq : tq + 1], u2[:], g_t[:, tq : tq + 1]
                    )
                    nc.vector.tensor_scalar_mul(
                        q2p8[:, tq, :], aqp[:, 0:D], s_vec[:, tq : tq + 1]
                    )
                yield  # aq done
                # p2q half-1 and coatt, interleaved; tail drains ping-pong
                # between DVE and the (by now idle) ACT engine
                emit_p2q_pair(2)
                nc.sync.dma_start(p2q_dst[:, 4:6, :], p2q_sb[:, 4:6, :])
                emit_co_pair(0, act=True)
                nc.sync.dma_start(co_dst[:, 0:2, :], co_sb[:, 0:2, :])
                emit_p2q_pair(3, act=(bi == 1))
                nc.sync.dma_start(p2q_dst[:, 6:8, :], p2q_sb[:, 6:8, :])
                emit_co_pair(1)
                nc.gpsimd.dma_start(co_dst[:, 2:4, :], co_sb[:, 2:4, :])
                emit_co_pair(2, act=True)
                nc.gpsimd.dma_start(co_dst[:, 4:6, :], co_sb[:, 4:6, :])
                emit_co_pair(3, split=True)
                nc.sync.dma_start(co_dst[:, 6:7, :], co_sb[:, 6:7, :])
                nc.gpsimd.dma_start(co_dst[:, 7:8, :], co_sb[:, 7:8, :])
                yield  # batch done

            # software-pipeline the two batches: batch 1's head and first
            # half slot in while batch 0 runs its attention tail.
            g0, g1 = emit_batch(0), emit_batch(1)
            next(g0)  # b0 head
            next(g0)  # b0 half0
            next(g1)  # b1 head
            next(g0)  # b0 half1
            next(g1)  # b1 half0
            next(g1)  # b1 half1
            next(g0)  # b0 aq
            next(g0)  # b0 tail -> exhausts below
            for _ in g0:
                pass
            for _ in g1:
                pass

    return nc


_nc_cache = None


def kernel(passage, question, passage_mask, question_mask, W, b):
    global _nc_cache
    _install_bir_wait_split()
    if _nc_cache is None:
        _nc_cache = build_nc()
    nc = _nc_cache

    passage = np.ascontiguousarray(passage, dtype=np.float32)
    question = np.ascontiguousarray(question, dtype=np.float32)
    passage_mask = np.ascontiguousarray(passage_mask, dtype=np.int32)
    question_mask = np.ascontiguousarray(question_mask, dtype=np.int32)
    W = np.ascontiguousarray(W, dtype=np.float32)
    b = np.ascontiguousarray(b, dtype=np.float32)

    in_maps = []
    for c in range(N_CORES):
        s = slice(c * NB, (c + 1) * NB)
        in_maps.append(
            {
                "passage": passage[s],
                "question": question[s],
                "passage_mask": passage_mask[s],
                "question_mask": question_mask[s],
                "W": W,
                "b": b,
            }
        )
    res = run_bass_kernel_spmd(nc, in_maps, list(range(N_CORES)))
    p2q = np.concatenate([r["p2q"] for r in res.results], axis=0)
    coatt = np.concatenate([r["coatt"] for r in res.results], axis=0)
    return p2q, coatt
